# revision 27
# baseline (speedup 1.0000x reference)
"""Trainium2 Bass kernel for fused MHA with q/k std-normalization.

Reference computation (per batch b, head h):
    q,k,v = x[b].T @ Wq/Wk/Wv          [T, 64] each
    q = (q - mean_e) / (std_e(ddof=1) + 1e-5)   (same for k)
    attn = softmax(q @ k.T / 8)
    out[b, h*64:(h+1)*64, :] = (attn @ v).T

Sharding: 8 cores = 4 batches x 2 half-head-groups. Core c handles batch
c//2, heads (c%2)*8 .. (c%2)*8+8. Fully head-independent, no collectives.

Per-core schedule: 8 local heads as 4 pairs, software-pipelined so the
Activation engine's exp stream (the serial bottleneck: 256 x [128,1024]
tiles) overlaps the PE projection of the next pair:

  window p: [attention strips of pair p-1] interleaved [projection pair p]

Engine assignment: PE matmuls/transposes; ACT does ONLY exp; DVE does all
PSUM evacuation + stats (bn_stats/bn_aggr on psum directly, rsqrt via
tensor_scalar pow(-0.5), eps 1e-5 dropped: relative effect ~1e-5).
q-hat staging is bf16 (halves transpose cost to 1 cyc/row; scores matmul
runs bf16 at the same 1 cyc/row as f32r). The q duplication onto both
partition halves (for the two tile_position score matmuls) happens inside
the PE transpose via a stride-0 broadcast access pattern.

Attention per head unchanged in structure: scores^T [s,t] via two
concurrent-row-group K=64 matmuls (kTi even/odd s-chunk partition halves),
exp with scale=1/8 folded (max-subtraction-free: |scores|/8 <= 7.94),
attn@V accumulates [v|1] so row 64 gives the softmax denominator; DVE
reciprocal + PE ones-broadcast + DVE multiply for the division.
"""

import sys

if "/opt/trn_rl_repo" not in sys.path:
    sys.path.insert(0, "/opt/trn_rl_repo")

import numpy as np

B, D, T, H = 4, 1024, 2048, 16
NHL = 8            # heads per core
NPAIR = NHL // 2   # head pairs per core
DH = 64            # head dim
NT = T // 128      # 16 t-tiles
ND = D // 128      # 8 d-chunks
NST = T // 512     # 4 t-strips

_prog = None


def _build(loop_n=None, part=None):
    import contextlib
    import concourse.bass as bass
    import concourse.bacc as bacc
    import concourse.tile as tile
    from concourse import mybir
    from concourse.masks import make_identity

    f32 = mybir.dt.float32
    f32r = mybir.dt.float32r
    bf16 = mybir.dt.bfloat16
    AF = mybir.ActivationFunctionType
    ALU = mybir.AluOpType

    nc = bacc.Bacc()
    x_ext = nc.dram_tensor("x_local", [NT, ND, 128, 128], bf16, kind="ExternalInput")
    w_ext = nc.dram_tensor("w_local", [ND, 128, NHL * 192], bf16, kind="ExternalInput")
    out_ext = nc.dram_tensor("out_local", [NHL * DH, T], f32, kind="ExternalOutput")

    with tile.TileContext(nc) as tc:
      with (tc.For_i(0, loop_n, 1) if loop_n else contextlib.nullcontext()):
        with tc.tile_pool(name="persist", bufs=1) as persist, \
             tc.tile_pool(name="xw", bufs=1) as xwp, \
             tc.tile_pool(name="qkT", bufs=1) as qkTp, \
             tc.tile_pool(name="vp", bufs=1) as vp, \
             tc.tile_pool(name="kst", bufs=2) as kstp, \
             tc.tile_pool(name="stage", bufs=4) as stpool, \
             tc.tile_pool(name="stats", bufs=8) as statp, \
             tc.tile_pool(name="pt", bufs=4) as ptp, \
             tc.tile_pool(name="osb", bufs=6) as osbp, \
             tc.tile_pool(name="outsb", bufs=2) as outp, \
             tc.tile_pool(name="dt", bufs=2) as dtp, \
             tc.tile_pool(name="pg", bufs=2, space="PSUM") as pgp, \
             tc.tile_pool(name="trp", bufs=1, space="PSUM") as trp, \
             tc.tile_pool(name="sp", bufs=2, space="PSUM") as spp, \
             tc.tile_pool(name="op", bufs=1, space="PSUM") as opp:

            identf = persist.tile([128, 128], f32, tag="identf")
            make_identity(nc, identf)
            identb = persist.tile([128, 128], bf16, tag="identb")
            nc.vector.tensor_copy(identb, identf)

            # weights then x, in consumption order. One batched DMA per
            # d-chunk / t-tile: per-tile DMAs serialize on the HWDGE
            # descriptor generator (~0.65us each) and pace the whole
            # first window otherwise.
            wsb = [xwp.tile([128, NHL * 192], bf16, tag=f"w{dc}", name=f"w{dc}")
                   for dc in range(ND)]
            for dc in range(ND):
                nc.sync.dma_start(out=wsb[dc], in_=w_ext[dc])
            xsb = [xwp.tile([128, ND, 128], bf16, tag=f"x{ti}",
                            name=f"x{ti}") for ti in range(NT)]
            for ti in range(NT):
                nc.sync.dma_start(
                    out=xsb[ti], in_=x_ext[ti].rearrange("dc p f -> p dc f"))

            # per-slot persistent head tensors (slot = (pair%2)*2 + m; two
            # pairs in flight). qTd: [128, T] bf16, q-hat^T duplicated on
            # both partition halves. kTi: [128, T/2] bf16, even s-chunks on
            # partitions 0:64, odd on 64:128. vt: [128, NT, 65] f32,
            # [:, :, 64] = 1 (softmax denominator through attn@V).
            qTd = [qkTp.tile([128, T], bf16, tag=f"qTd{s}", name=f"qTd{s}")
                   for s in range(4)]
            kTi = [qkTp.tile([128, T // 2], bf16, tag=f"kTi{s}",
                             name=f"kTi{s}") for s in range(4)]
            vt = [vp.tile([128, NT, 65], f32r, tag=f"vt{s}", name=f"vt{s}")
                  for s in range(4)]
            for s in range(4):
                nc.vector.memset(vt[s][:, :, 64:65].bitcast(f32), 1.0)

            kstg_live = {}

            def proj_unit(p, ti):
                """Project q,k,v of head pair p for one t-tile; normalize;
                stage transposed q/k and v. Generator: yields after each PE
                op so the caller can interleave attention work."""
                pg = pgp.tile([128, 384], f32, tag="pg")
                for dc in range(ND):
                    nc.tensor.matmul(
                        pg, lhsT=xsb[ti][:, dc],
                        rhs=wsb[dc][:, p * 384: (p + 1) * 384],
                        start=(dc == 0), stop=(dc == ND - 1))
                    yield
                # one psum tile holds all of this unit's transposes (q of
                # both heads + k of both on odd tiles) so they pipeline
                # without waiting on each other's evacuation
                trt = trp.tile([128, 4, 128], bf16, tag="tr")
                for m in range(2):
                    h = 2 * p + m
                    slot = (p % 2) * 2 + m
                    base = m * 192
                    st6 = statp.tile([128, 2, 6], f32, tag="st6")
                    nc.vector.bn_stats(st6[:, 0], pg[:, base: base + 64])
                    nc.vector.bn_stats(st6[:, 1], pg[:, base + 64: base + 128])
                    mv = statp.tile([128, 2, 2], f32, tag="mv")
                    nc.vector.bn_aggr(mv[:, 0], st6[:, 0])
                    nc.vector.bn_aggr(mv[:, 1], st6[:, 1])
                    # inv = rsqrt(var*64/63) (unbiased std; eps dropped:
                    # relative effect ~1e-5). ACT Sqrt would thrash the
                    # activation table against the exp stream (1.3us/load),
                    # so compute rsqrt on DVE: quake seed + one Newton step
                    # (~0.2% max err, well under the bf16 noise floor).
                    # The 64/63 folds into the magic and the Newton scale.
                    i32 = mybir.dt.int32
                    ii = statp.tile([128, 2], i32, tag="ii")
                    nc.vector.tensor_scalar(
                        out=ii, in0=mv[:, :, 1].bitcast(i32), scalar1=1,
                        scalar2=None, op0=ALU.logical_shift_right)
                    y0b = statp.tile([128, 2], i32, tag="y0b")
                    nc.vector.tensor_scalar(
                        out=y0b, in0=ii, scalar1=-1, scalar2=0x5F3759DF - 95292,
                        op0=ALU.mult, op1=ALU.add)
                    hx = statp.tile([128, 2], f32, tag="hx")
                    nc.vector.tensor_scalar(
                        out=hx, in0=mv[:, :, 1], scalar1=float(0.5 * 64 / 63),
                        scalar2=None, op0=ALU.mult)
                    y0 = y0b.bitcast(f32)
                    y2t = statp.tile([128, 2], f32, tag="y2t")
                    nc.vector.tensor_tensor(out=y2t, in0=y0, in1=y0,
                                            op=ALU.mult)
                    nc.vector.tensor_tensor(out=y2t, in0=hx, in1=y2t,
                                            op=ALU.mult)
                    nc.vector.tensor_scalar(
                        out=y2t, in0=y2t, scalar1=-1.0, scalar2=1.5,
                        op0=ALU.mult, op1=ALU.add)
                    inv = statp.tile([128, 2], f32, tag="inv")
                    nc.vector.tensor_tensor(out=inv, in0=y0, in1=y2t,
                                            op=ALU.mult)
                    # qhat staged bf16; cheap SBUF bf16 copy duplicates it
                    # into the second col-half (2x/4x DVE mode) so one
                    # [128,128] transpose fills both partition halves of qTd
                    qstg = stpool.tile([128, 128], bf16, tag="qstg")
                    nc.vector.tensor_scalar(
                        out=qstg[:, 0:64], in0=pg[:, base: base + 64],
                        scalar1=mv[:, 0, 0:1], scalar2=inv[:, 0:1],
                        op0=ALU.subtract, op1=ALU.mult)
                    nc.vector.tensor_copy(qstg[:, 64:128], qstg[:, 0:64])
                    if ti % 2 == 0:
                        kstg = kstp.tile([128, 2, 64], bf16, tag=f"kstg{slot}",
                                         name=f"kstg{slot}")
                        kstg_live[slot] = kstg
                    else:
                        kstg = kstg_live[slot]
                    nc.vector.tensor_scalar(
                        out=kstg[:, ti % 2], in0=pg[:, base + 64: base + 128],
                        scalar1=mv[:, 1, 0:1], scalar2=inv[:, 1:2],
                        op0=ALU.subtract, op1=ALU.mult)
                    # v evacuation: ACT in the fill window (no exp yet, DVE
                    # is the pacer there), DVE in steady windows (ACT busy
                    # with the previous pair's exp).
                    vcopy = nc.scalar.copy if p == 0 else nc.vector.tensor_copy
                    vcopy(vt[slot][:, ti, 0:64], pg[:, base + 128: base + 192])
                    # transpose qhat with stride-0 dup into both halves.
                    # In window 0 (pair 0) nothing runs on ACT yet, so the
                    # psum evacuations go there; later windows keep them on
                    # DVE (ACT is busy with exp).
                    evac = nc.scalar.copy if p == 0 else nc.vector.tensor_copy
                    nc.tensor.transpose(trt[:, m], qstg, identb)
                    evac(qTd[slot][:, ti * 128: (ti + 1) * 128], trt[:, m])
                    yield
                    if ti % 2 == 1:
                        nc.tensor.transpose(
                            trt[:, 2 + m], kstg.rearrange("p a f -> p (a f)"),
                            identb)
                        evac(kTi[slot][:, (ti // 2) * 128:
                                       (ti // 2) * 128 + 128], trt[:, 2 + m])
                        yield

            def attn_strip(p, m, st, osbs, pump):
                """One 512-wide t-strip of attention for head pair p, member
                m. Accumulates [v|1]^T @ exp(scores) into op psum; stashes
                the strip output + denominator row in osbs."""
                h = 2 * p + m
                slot = (p % 2) * 2 + m
                op_ps = opp.tile([65, 512], f32, tag="op")
                NJ = 8
                LOOKAHEAD = 2
                sps, pts = [], []

                def emit_scores(j):
                    sp = spp.tile([128, 1024], f32, tag="sp", name=f"sp{j}")
                    for u in range(2):
                        hb = u * 64
                        nc.tensor.matmul(
                            sp[:, u * 512: (u + 1) * 512],
                            lhsT=kTi[slot][hb: hb + 64,
                                           j * 128: (j + 1) * 128],
                            rhs=qTd[slot][hb: hb + 64,
                                          st * 512: (st + 1) * 512],
                            start=True, stop=True,
                            tile_position=(hb, 0))
                    pt = ptp.tile([128, 1024], f32r, tag="pt", name=f"pt{j}")
                    nc.scalar.activation(pt, sp, AF.Exp, scale=0.125)
                    sps.append(sp)
                    pts.append(pt)

                def emit_pv(j):
                    for u in range(2):
                        sc = 2 * j + u
                        nc.tensor.matmul(
                            op_ps, lhsT=vt[slot][:, sc],
                            rhs=pts[j][:, u * 512: (u + 1) * 512],
                            start=(sc == 0), stop=(sc == 15))

                for j in range(LOOKAHEAD):
                    emit_scores(j)
                for j in range(NJ):
                    if j + LOOKAHEAD < NJ:
                        emit_scores(j + LOOKAHEAD)
                    emit_pv(j)
                    pump(3)
                osb = osbp.tile([65, 512], f32, tag="osb")
                nc.vector.tensor_copy(osb, op_ps)
                osbs.append(osb)

            def head_divide(p, m, osbs):
                """Divide the 4 strip outputs of head (p, m) by the softmax
                denominators and DMA to the output. The denominator row is
                broadcast across partitions by GpSimd (idle engine), keeping
                the PE out of the division entirely."""
                h = 2 * p + m
                for st in range(NST):
                    rb = dtp.tile([1, 512], f32, tag="rb")
                    nc.vector.reciprocal(rb, osbs[st][64:65, :])
                    rbb = dtp.tile([64, 512], f32, tag="rbb")
                    nc.gpsimd.partition_broadcast(rbb, rb)
                    outt = outp.tile([64, 512], f32, tag="outt")
                    nc.vector.tensor_mul(outt, osbs[st][0:64, :], rbb)
                    nc.sync.dma_start(
                        out=out_ext[h * 64: (h + 1) * 64,
                                    st * 512: (st + 1) * 512],
                        in_=outt)

            # software pipeline: window p runs attention strips from the
            # ready queue with projection ops of pair p pumped into the PE
            # gaps (the exp stream on ACT outpaces the attention matmuls).
            # After a pair's projection drains, its first two strips run at
            # the window tail, which levels the exp load into the fill and
            # shrinks the ACT-bound final window.
            from collections import deque

            strip_queue = deque()
            osbs_map = {}

            def run_strip(item, pump):
                p, m, st = item
                osbs = osbs_map.setdefault((p, m), [])
                attn_strip(p, m, st, osbs, pump)
                if st == NST - 1:
                    head_divide(p, m, osbs)

            for p in range(NPAIR + 1):
                if p < NPAIR:
                    def proj_stream(p=p):
                        for ti in range(NT):
                            yield from proj_unit(p, ti)
                    gen = proj_stream()
                else:
                    gen = None

                def pump(n, gen=gen):
                    if gen is not None:
                        for _ in range(n):
                            if next(gen, "done") == "done":
                                break

                nstrips = 6 if 0 < p < NPAIR else 8
                for s in range(nstrips):
                    if not strip_queue:
                        break
                    run_strip(strip_queue.popleft(), pump)
                pump(10 ** 9)
                if p < NPAIR:
                    strip_queue.extend(
                        (p, m, st) for m in range(2) for st in range(NST))
                    for _ in range(2):       # pull the new pair's first
                        run_strip(strip_queue.popleft(), lambda n: None)
            while strip_queue:
                run_strip(strip_queue.popleft(), lambda n: None)
    nc.finalize()
    return nc


def _get_prog():
    global _prog
    if _prog is None:
        _prog = _build()
    return _prog


def make_in_maps(x, qkv):
    import ml_dtypes

    bf = ml_dtypes.bfloat16
    x = np.ascontiguousarray(np.asarray(x, dtype=np.float32))
    qkv = np.ascontiguousarray(np.asarray(qkv, dtype=np.float32))
    in_maps = []
    for c in range(8):
        b = c // 2
        hs = slice((c % 2) * 8, (c % 2) * 8 + 8)
        # [16 ti, 8 dc, 128 dp, 128 tf]
        xp = (x[b].reshape(ND, 128, NT, 128).transpose(2, 0, 1, 3)
              .astype(bf).copy())
        # [8 dc, 128 dp, h*192 + n*64 + e]
        wp = (qkv[:, hs].transpose(2, 1, 0, 3)
              .reshape(D, NHL * 192).reshape(ND, 128, NHL * 192)
              .astype(bf).copy())
        in_maps.append({"x_local": xp, "w_local": wp})
    return in_maps


def gather(results):
    out = np.empty((B, D, T), np.float32)
    for c in range(8):
        out[c // 2, (c % 2) * 512: (c % 2) * 512 + 512, :] = \
            results[c]["out_local"]
    return out


def kernel(**inputs):
    from concourse.bass_utils import run_bass_kernel_spmd

    nc = _get_prog()
    in_maps = make_in_maps(inputs["x"], inputs["qkv"])
    res = run_bass_kernel_spmd(nc, in_maps, list(range(8)))
    return gather(res.results)
